# revision 114
# baseline (speedup 1.0000x reference)
"""GQA attention (B=2, S=2048, DM=1024, H=16, KH=4, RoPE, causal) on 8 TRN2 cores.

Sharding: DP=2 over batch x TP=4 over heads. Core c handles batch c//4 and
q-heads [4r, 4r+4), kv-head r, where r = c % 4. Each core computes a partial
out^T = wo_shard @ attn_shard of shape [DM, S] in bf16; the host sums the 4
partials per batch in f32 and transposes (gather/unshard).

Per-core kernel (single NEFF, SPMD):
  - Q/K computed feature-major ([feat, tok]) via transposed weight layouts
    prepared on host; V computed feature-major then PE-transposed to
    token-major with a ones-column appended (rowsum trick).
  - RoPE: scalar_tensor_tensor fuses the PSUM evacuation with the cos
    multiply; adjacent-partition swap via stream_shuffle; bf16 tables.
  - Causal softmax without max-subtraction (logits bounded ~|2.5|); exp on
    ACT over two heads at once; causal mask as a post-exp bf16 multiply on
    only the 128-wide triangular sub-block of each diagonal key-block.
  - Scores matmuls run two heads concurrently in the two 64-row PE groups.
  - x loaded with one 1 MiB DMA per chunk; output staged per chunk in bf16
    and stored with one 1 MiB DMA.
"""

import numpy as np
import ml_dtypes

import concourse.bass as bass
import concourse.mybir as mybir
import concourse.tile as tile
from concourse import bacc
from concourse.bass_utils import run_bass_kernel_spmd
from concourse.masks import make_identity

F32 = mybir.dt.float32
F32R = mybir.dt.float32r
BF16 = mybir.dt.bfloat16

B, S, DM, H, KH, HD = 2, 2048, 1024, 16, 4, 64
N_CORES = 8
TPG = 4                 # tensor-parallel group size
QH = H // TPG           # q-heads per core
KFEAT = QH * HD         # 256 q-features per core
SC = 512                # token chunk
NCH = S // SC           # 4
KB = 128                # key block
NKB = S // KB           # 16
SCALE = 1.0 / np.sqrt(HD)
XOR1 = [i ^ 1 for i in range(32)]
# Schraudolph exp in bf16-bit space: bitcast_bf16(int16(s*SCH_A + SCH_B))
# approximates exp(SCALE*s) to ~3.3% max rel err (calibrated offline)
SCH_A = SCALE * np.log2(np.e) * 128.0
SCH_B = 16250.64
MASK_PE = False          # apply causal mask via PE accumulate vs DVE multiply
WARMUP = True            # HAM warmup burst at kernel start

LAST_RESULTS = None     # BassKernelResults of the most recent run (for test.py)
_NC_CACHE = None


def build_nc():
    nc = bacc.Bacc("TRN2", target_bir_lowering=False, debug=False,
                   num_devices=N_CORES)

    xT = nc.declare_dram_parameter("xT", [DM, S], BF16, isOutput=False)
    # weights packed into two tensors on one queue: wq alone first (it gates
    # the first projection), then wkv+wo
    # wpack: [:, 0:2048] wq as [8 kb, 256]
    # wpack2: [:, 0:1024] wkv as [8 kb, 128], [:, 1024:3072] wo as [2 c, 1024]
    wpackD = nc.declare_dram_parameter("wpack", [128, 2048], BF16,
                                       isOutput=False)
    wpack2D = nc.declare_dram_parameter("wpack2", [128, 3072], BF16,
                                        isOutput=False)
    # rope tables + causal mask packed: [:, 0:2048] cos, [:, 2048:4096] sin,
    # [:, 4096:4224] additive triangle mask (-240 where k > c)
    rpackD = nc.declare_dram_parameter("rpack", [128, 4224], BF16,
                                       isOutput=False)
    out = nc.declare_dram_parameter("out", [DM, S], BF16, isOutput=True)

    xT_v = xT.rearrange("(kb p) n -> p kb n", p=128)        # [128, 8, S]
    out_v = out.rearrange("(mb p) n -> p mb n", p=128)      # [128, 8, S]

    EXP = mybir.ActivationFunctionType.Exp
    MUL = bass.mybir.AluOpType.mult
    ADD = bass.mybir.AluOpType.add

    with tile.TileContext(nc) as tc:
        with (
            tc.tile_pool(name="consts", bufs=1) as consts,
            tc.tile_pool(name="kch", bufs=NCH) as kch_pool,
            tc.tile_pool(name="qch", bufs=NCH) as qch_pool,
            tc.tile_pool(name="ach", bufs=2 * NCH) as ach_pool,
            tc.tile_pool(name="v1p", bufs=NKB) as v1_pool,
            tc.tile_pool(name="xch", bufs=NCH) as xch_pool,
            tc.tile_pool(name="tmp", bufs=6) as tmp_pool,
            tc.tile_pool(name="pp", bufs=8) as p_pool,
            tc.tile_pool(name="rp", bufs=8) as r_pool,
            tc.tile_pool(name="ocp", bufs=4) as oc_pool,
            tc.tile_pool(name="outb", bufs=2) as outb_pool,
            tc.tile_pool(name="acc", bufs=2, space="PSUM") as acc_pool,
            tc.tile_pool(name="oac", bufs=1, space="PSUM") as oacc_pool,
            tc.tile_pool(name="sme", bufs=2, space="PSUM") as s_pool,
        ):
            # ---- constants ----
            wpack = consts.tile([128, 2048], BF16, tag="wpack")
            wpack2 = consts.tile([128, 3072], BF16, tag="wpack2")
            rpack = consts.tile([128, 4224], BF16, tag="rpack")
            ident = consts.tile([128, 128], BF16, tag="ident")

            def wq_ap(kb, lo, hi):
                return wpack[:, kb * 256 + lo: kb * 256 + hi]

            def wkv_ap(kb):
                return wpack2[:, kb * 128: (kb + 1) * 128]

            def wo_ap(c, lo, hi):
                base = 1024 + c * 1024
                return wpack2[:, base + lo: base + hi]

            def cos_ap(npart, cols):
                return rpack[0:npart, cols]

            def sin_ap(npart, cols):
                return rpack[0:npart, 2048 + cols.start: 2048 + cols.stop]

            def mask_ap():
                return rpack[:, 4096:4224]

            # spread the prologue loads over three DMA queues (sync, scalar
            # HWDGE, gpsimd SWDGE) so they don't serialize on one engine,
            # and keep the DMA count below the 8 semaphore lanes to avoid
            # false lane-reuse dependencies
            nc.scalar.dma_start(wpack[:], wpackD[:])
            nc.scalar.dma_start(wpack2[:], wpack2D[:])
            nc.gpsimd.dma_start(rpack[:], rpackD[:])
            make_identity(nc, ident[:])

            # ~3.5us of dummy back-to-back matmuls while the first DMAs are
            # in flight: flips the PE HAM clock-gate to 8/8 (2.4 GHz) before
            # the first projection, instead of running the first ~20us of
            # real matmuls at the cold 1.2 GHz default.
            if WARMUP:
                warm_ps = acc_pool.tile([128, 128], F32, tag="acc")
                for _ in range(32):
                    nc.tensor.matmul(warm_ps[:], ident[:], ident[:],
                                     start=True, stop=True)

            K_ch = []       # per-chunk K, feature-major, duplicated rows
            Q_ch = {}       # per-chunk Q, feature-major, [128, 2, SC]
            A_ch = {}       # per-chunk attn output, feature-major
            V1_kb = []      # per key-block token-major [V | 1]

            def proj_load(c0):
                cols = slice(c0 * SC, (c0 + 1) * SC)
                xc = xch_pool.tile([128, 8, SC], BF16, tag="x",
                                   name=f"x_c{c0}")
                nc.sync.dma_start(xc[:], xT_v[:, :, cols])
                return xc

            def rope(dst, src_ps, cols, npart):
                """dst[bf16] = RoPE(src_ps[f32 PSUM]) on npart partitions."""
                qsw = tmp_pool.tile([npart, SC], F32, tag="qsw")
                t1 = tmp_pool.tile([npart, SC], BF16, tag="t1")
                t2 = tmp_pool.tile([npart, SC], BF16, tag="t2")
                nc.vector.stream_shuffle(qsw[:], src_ps, XOR1)
                # fused PSUM evacuation + cos multiply
                nc.vector.scalar_tensor_tensor(
                    t1[:], src_ps, 0.0, cos_ap(npart, cols), ADD, MUL)
                nc.vector.tensor_tensor(t2[:], qsw[:], sin_ap(npart, cols), MUL)
                nc.vector.tensor_tensor(dst, t1[:], t2[:], ADD)

            def proj_q(c0, xc, ms=(0, 1), q_sb=None):
                cols = slice(c0 * SC, (c0 + 1) * SC)
                if q_sb is None:
                    q_sb = qch_pool.tile([128, 2, SC], BF16, tag="q")
                    Q_ch[c0] = q_sb
                # Q projection + RoPE, two 128-feature tiles (2 heads each)
                for m in ms:
                    q_ps = acc_pool.tile([128, SC], F32, tag="acc")
                    for kb in range(8):
                        nc.tensor.matmul(
                            q_ps[:],
                            wq_ap(kb, m * 128, (m + 1) * 128),
                            xc[:, kb, :],
                            start=(kb == 0), stop=(kb == 7),
                        )
                    rope(q_sb[:, m, :], q_ps[:], cols, 128)
                return q_sb

            def proj_kv(c0, xc):
                cols = slice(c0 * SC, (c0 + 1) * SC)
                k_sb = kch_pool.tile([128, SC], BF16, tag="k")
                K_ch.append(k_sb)
                # K (rows 0:64) and V (rows 64:128) projection
                kv_ps = acc_pool.tile([128, SC], F32, tag="acc")
                for kb in range(8):
                    nc.tensor.matmul(
                        kv_ps[:],
                        wkv_ap(kb),
                        xc[:, kb, :],
                        start=(kb == 0), stop=(kb == 7),
                    )
                # V cast FIRST on the DVE so the PE transposes (queued ahead
                # of the next projections in the FIFO) unblock before the
                # K rope finishes
                vtmp = tmp_pool.tile([128, SC], BF16, tag="vtmp")
                nc.vector.tensor_copy(vtmp[64:128, :], kv_ps[64:128, :])
                rope(k_sb[0:64, :], kv_ps[0:64, :], cols, 64)
                # duplicate K into partitions 64:128 (second PE row group)
                nc.sync.dma_start(k_sb[64:128, :], k_sb[0:64, :])
                for tb in range(4):
                    v1 = v1_pool.tile([128, 66], BF16, tag="v1")
                    V1_kb.append(v1)
                    vt_ps = acc_pool.tile([128, 64], BF16, tag="acc")
                    nc.tensor.transpose(
                        vt_ps[:], vtmp[64:128, tb * 128:(tb + 1) * 128],
                        ident[64:128, 64:128],
                    )
                    nc.vector.tensor_copy(v1[:, 0:64], vt_ps[:])
                    nc.vector.memset(v1[:, 64:65], 1.0)

            def attention_pair(c0, p, exp_dve=False):
                nkb = 4 * (c0 + 1)
                op2 = oacc_pool.tile([65, 2, SC], F32, tag="oacc")
                o0 = op2[:, 0, :]
                o1 = op2[:, 1, :]

                def av(kb, p2v, q0):
                    nc.tensor.matmul(
                        o0[:, q0:], V1_kb[kb][:, 0:65], p2v(0, q0),
                        start=(kb == 0), stop=(kb == nkb - 1),
                    )
                    nc.tensor.matmul(
                        o1[:, q0:], V1_kb[kb][:, 0:65], p2v(1, q0),
                        start=(kb == 0), stop=(kb == nkb - 1),
                    )

                for kb in range(nkb):
                    kc = K_ch[kb // 4]
                    kcols = slice((kb % 4) * 128, (kb % 4 + 1) * 128)
                    s2 = s_pool.tile([128, 2, SC], F32, tag="s2")
                    nc.tensor.matmul(
                        s2[:, 0, :],
                        kc[0:64, kcols],
                        Q_ch[c0][0:64, p, :],
                        start=True, stop=True,
                    )
                    nc.tensor.matmul(
                        s2[:, 1, :],
                        kc[64:128, kcols],
                        Q_ch[c0][64:128, p, :],
                        start=True, stop=True,
                        tile_position=(64, 0),
                    )
                    # on diagonal blocks, only q-cols >= q0 are live
                    j = kb - (nkb - 4)
                    q0 = 128 * j if j >= 0 else 0
                    if j >= 0 and MASK_PE:
                        # additive causal mask on the triangular 128-col
                        # sub-block via PE accumulate (cols >= q0+128 of this
                        # key block are fully causal-valid)
                        for h in range(2):
                            nc.tensor.matmul(
                                s2[:, h, q0:q0 + 128],
                                ident[:], mask_ap(),
                                start=False, stop=True, skip_group_check=True,
                            )
                    if exp_dve and kb % 2 == 1 and j < 0:
                        # Schraudolph exp on DVE: bf16 bits of exp(SCALE*s)
                        # via int16 linear map (splits the exp stream between
                        # the saturated ACT engine and the DVE)
                        p2i = p_pool.tile([128, 2, SC], mybir.dt.int16,
                                          tag="p2")
                        nc.vector.tensor_scalar(
                            p2i[:, :, q0:], s2[:, :, q0:],
                            SCH_A, SCH_B, MUL, ADD)

                        def p2v(sl_h, lo, hi=SC):
                            return p2i[:, sl_h, lo:hi].bitcast(BF16)
                    else:
                        p2 = p_pool.tile([128, 2, SC], BF16, tag="p2")
                        nc.scalar.activation(p2[:, :, q0:], s2[:, :, q0:],
                                             EXP, scale=SCALE)

                        def p2v(sl_h, lo, hi=SC):
                            return p2[:, sl_h, lo:hi]
                    if j >= 0 and not MASK_PE:
                        nc.vector.tensor_tensor(
                            p2v(slice(None), q0, q0 + 128),
                            p2v(slice(None), q0, q0 + 128),
                            mask_ap().unsqueeze(1).to_broadcast([128, 2, 128]),
                            MUL)
                    av(kb, p2v, q0)
                # evacuate PSUM accumulator to SBUF bf16 (frees the banks).
                # ACT does this: it reads PSUM faster than the DVE and the
                # vector engine is the more loaded of the two.
                oc2 = oc_pool.tile([65, 2, SC], BF16, tag="oc")
                nc.scalar.copy(oc2[:], op2[:])
                return oc2

            def divide_pair(a_tile, oc2):
                # reshape the [1, 2*512] sums row to [64, 16] so the
                # reciprocal runs on many DVE lanes instead of one
                rsum = r_pool.tile([64, 16], BF16, tag="rsum")
                nc.sync.dma_start(
                    rsum[:],
                    oc2[64:65, :, :].rearrange("o hh (a n) -> o (hh a) n", a=32))
                rrecf = r_pool.tile([64, 16], F32, tag="rrecf")
                nc.vector.reciprocal(rrecf[:], rsum[:])
                rrecs = r_pool.tile([64, 16], BF16, tag="rrecs")
                nc.vector.tensor_copy(rrecs[:], rrecf[:])
                rrec = r_pool.tile([1, 2, SC], BF16, tag="rrec")
                nc.sync.dma_start(
                    rrec[0:1, :, :].rearrange("o hh (a n) -> o (hh a) n", a=32),
                    rrecs[:])
                # one broadcast covers both heads' reciprocal rows: the
                # second head's multiply no longer waits on a second
                # gpsimd launch
                bc = r_pool.tile([64, 2, SC], BF16, tag="bc")
                nc.gpsimd.partition_broadcast(bc[:], rrec[0:1, :, :])
                nc.vector.tensor_tensor(
                    a_tile[0:64, :], oc2[0:64, 0, :], bc[:, 0, :], MUL)
                tb = r_pool.tile([64, SC], BF16, tag="tb")
                nc.vector.tensor_tensor(
                    tb[:], oc2[0:64, 1, :], bc[:, 1, :], MUL)
                # move to partitions 64:128 (DMA crosses partitions)
                nc.sync.dma_start(a_tile[64:128, :], tb[:])

            def out_proj(c0):
                ncols = slice(c0 * SC, (c0 + 1) * SC)
                ob = outb_pool.tile([128, 8, SC], BF16, tag="ob")
                for mb in range(8):
                    o_ps = acc_pool.tile([128, SC], F32, tag="acc")
                    for c in range(2):
                        nc.tensor.matmul(
                            o_ps[:],
                            wo_ap(c, mb * 128, (mb + 1) * 128),
                            A_ch[c0][c][:, :],
                            start=(c == 0), stop=(c == 1),
                        )
                    nc.vector.tensor_copy(ob[:, mb, :], o_ps[:])
                    if mb == 3:
                        nc.sync.dma_start(out_v[:, 0:4, ncols], ob[:, 0:4, :])
                nc.sync.dma_start(out_v[:, 4:8, ncols], ob[:, 4:8, :])

            X_ch = [proj_load(c) for c in range(NCH)]
            # chunk 0 prologue: emit the m0 Q-tile first, then the KV
            # projection, so the PE works on KV while the DVE ropes m0 and
            # attention(0, 0) (which only needs m0 + K + V) starts early
            q0_sb = proj_q(0, X_ch[0], ms=(0,))
            proj_kv(0, X_ch[0])
            proj_q(0, X_ch[0], ms=(1,), q_sb=q0_sb)
            pending_outproj = None
            for c0 in range(NCH):
                a_pair = [ach_pool.tile([128, SC], BF16, tag="a",
                                        name=f"a_c{c0}p{ii}")
                          for ii in range(2)]
                A_ch[c0] = a_pair
                oc0 = attention_pair(c0, 0)
                divide_pair(a_pair[0], oc0)
                if c0 + 1 < NCH:
                    proj_q(c0 + 1, X_ch[c0 + 1])  # fills PE bubbles
                elif pending_outproj is not None:
                    out_proj(pending_outproj)     # last chunk: drain backlog
                    pending_outproj = None
                oc1 = attention_pair(c0, 1, exp_dve=(c0 == NCH - 1))
                if c0 + 1 < NCH:
                    proj_kv(c0 + 1, X_ch[c0 + 1])
                else:
                    # tail keep-warm: hold the PE HAM at 8/8 through the
                    # divide/out-proj drain so the final matmuls run at
                    # 2.4 GHz. Reading oc1 makes these matmuls depend on the
                    # final evacuation, pinning them into the drain window
                    # (ident-only dummies would be scheduled much earlier).
                    warm2 = acc_pool.tile([128, SC], F32, tag="acc")
                    for _ in range(20):
                        nc.tensor.matmul(warm2[:], ident[0:65, :],
                                         oc1[0:65, 0, :],
                                         start=True, stop=True)
                divide_pair(a_pair[1], oc1)
                if pending_outproj is not None:
                    out_proj(pending_outproj)
                pending_outproj = c0
            out_proj(NCH - 1)

    nc.compile()
    return nc


def shard_inputs(x, wq, wk, wv, wo, freqs_cos, freqs_sin):
    """Build the 8 per-core input maps (host-side layout prep)."""
    bf = ml_dtypes.bfloat16
    x = np.ascontiguousarray(np.asarray(x, dtype=np.float32))
    wq = np.asarray(wq, dtype=np.float32)
    wk = np.asarray(wk, dtype=np.float32)
    wv = np.asarray(wv, dtype=np.float32)
    wo = np.asarray(wo, dtype=np.float32)
    cos = np.asarray(freqs_cos, dtype=np.float32)   # [S, 32]
    sin = np.asarray(freqs_sin, dtype=np.float32)

    rope_cos = np.repeat(cos.T, 2, axis=0)          # [64, S]
    rope_sin = np.repeat(sin.T, 2, axis=0)
    rope_sin[0::2, :] *= -1.0                       # row 2i: -sin_i, 2i+1: +sin_i

    # causal mask for the triangular diagonal sub-block: additive (-240 where
    # k > c; exp underflows to 0) for the PE path, multiplicative otherwise
    kk = np.arange(128)[:, None]
    cc = np.arange(128)[None, :]
    if MASK_PE:
        mask01 = np.where(kk > cc, -240.0, 0.0)
    else:
        mask01 = (kk <= cc).astype(np.float32)

    # rpack: [cos | sin | mask], each table duplicated to 128 partitions
    rpack = np.concatenate([
        np.tile(rope_cos, (2, 1)),                  # [128, S]
        np.tile(rope_sin, (2, 1)),                  # [128, S]
        mask01,                                     # [128, 128]
    ], axis=1).astype(bf)

    in_maps = []
    for core in range(N_CORES):
        b, r = divmod(core, TPG)
        xT = np.ascontiguousarray(x[b].T)                         # [DM, S]
        wq_s = wq[r * KFEAT:(r + 1) * KFEAT]                      # [256, DM]
        wk_s = wk[r * HD:(r + 1) * HD]                            # [64, DM]
        wv_s = wv[r * HD:(r + 1) * HD]
        wkvT = np.concatenate([wk_s, wv_s], axis=0).T             # [DM, 128]
        wqT = wq_s.T                                              # [DM, 256]
        woT = wo[:, r * KFEAT:(r + 1) * KFEAT].T                  # [256, DM]
        # wq [128, 8kb, 256]; wkv [128, 8kb, 128]; wo [128, 2c, 1024]
        wq_l = wqT.reshape(8, 128, KFEAT).transpose(1, 0, 2).reshape(128, 2048)
        wkv_l = wkvT.reshape(8, 128, 128).transpose(1, 0, 2).reshape(128, 1024)
        wo_l = woT.reshape(2, 128, DM).transpose(1, 0, 2).reshape(128, 2048)
        wpack2 = np.concatenate([wkv_l, wo_l], axis=1)            # [128, 3072]
        in_maps.append({
            "xT": xT.astype(bf),
            "wpack": np.ascontiguousarray(wq_l).astype(bf),
            "wpack2": np.ascontiguousarray(wpack2).astype(bf),
            "rpack": rpack,
        })
    return in_maps


def unshard(results):
    """Sum TP partials per batch and transpose back to [B, S, DM]."""
    out = np.empty((B, S, DM), dtype=np.float32)
    for b in range(B):
        acc = results[b * TPG]["out"].astype(np.float32)
        for r in range(1, TPG):
            acc = acc + results[b * TPG + r]["out"].astype(np.float32)
        out[b] = acc.T
    return out


def kernel(**inputs):
    global LAST_RESULTS, _NC_CACHE
    if _NC_CACHE is None:
        _NC_CACHE = build_nc()
    in_maps = shard_inputs(**inputs)
    LAST_RESULTS = run_bass_kernel_spmd(_NC_CACHE, in_maps, list(range(N_CORES)))
    return unshard(LAST_RESULTS.results)


# revision 115
# speedup vs baseline: 1.1667x; 1.1667x over previous
"""GQA attention (B=2, S=2048, DM=1024, H=16, KH=4, RoPE, causal) on 8 TRN2 cores.

Sharding: DP=2 over batch x TP=4 over heads. Core c handles batch c//4 and
q-heads [4r, 4r+4), kv-head r, where r = c % 4. Each core computes a partial
out^T = wo_shard @ attn_shard of shape [DM, S] in bf16; the host sums the 4
partials per batch in f32 and transposes (gather/unshard).

Per-core kernel (single NEFF, SPMD):
  - Q/K computed feature-major ([feat, tok]) via transposed weight layouts
    prepared on host; V computed feature-major then PE-transposed to
    token-major with a ones-column appended (rowsum trick).
  - RoPE: scalar_tensor_tensor fuses the PSUM evacuation with the cos
    multiply; adjacent-partition swap via stream_shuffle; bf16 tables.
  - Causal softmax without max-subtraction (logits bounded ~|2.5|); exp on
    ACT over two heads at once; causal mask as a post-exp bf16 multiply on
    only the 128-wide triangular sub-block of each diagonal key-block.
  - Scores matmuls run two heads concurrently in the two 64-row PE groups.
  - x loaded with one 1 MiB DMA per chunk; output staged per chunk in bf16
    and stored with one 1 MiB DMA.
"""

import numpy as np
import ml_dtypes

import concourse.bass as bass
import concourse.mybir as mybir
import concourse.tile as tile
from concourse import bacc
from concourse.bass_utils import run_bass_kernel_spmd
from concourse.masks import make_identity

F32 = mybir.dt.float32
F32R = mybir.dt.float32r
BF16 = mybir.dt.bfloat16

B, S, DM, H, KH, HD = 2, 2048, 1024, 16, 4, 64
N_CORES = 8
TPG = 4                 # tensor-parallel group size
QH = H // TPG           # q-heads per core
KFEAT = QH * HD         # 256 q-features per core
SC = 512                # token chunk
NCH = S // SC           # 4
KB = 128                # key block
NKB = S // KB           # 16
SCALE = 1.0 / np.sqrt(HD)
XOR1 = [i ^ 1 for i in range(32)]
# Schraudolph exp in bf16-bit space: bitcast_bf16(int16(s*SCH_A + SCH_B))
# approximates exp(SCALE*s) to ~3.3% max rel err (calibrated offline)
SCH_A = SCALE * np.log2(np.e) * 128.0
SCH_B = 16250.64
MASK_PE = False          # apply causal mask via PE accumulate vs DVE multiply
WARMUP = True            # HAM warmup burst at kernel start

LAST_RESULTS = None     # BassKernelResults of the most recent run (for test.py)
_NC_CACHE = None


def build_nc():
    nc = bacc.Bacc("TRN2", target_bir_lowering=False, debug=False,
                   num_devices=N_CORES)

    xT = nc.declare_dram_parameter("xT", [DM, S], BF16, isOutput=False)
    # weights packed into two tensors on one queue: wq alone first (it gates
    # the first projection), then wkv+wo
    # wpack: [:, 0:2048] wq as [8 kb, 256]
    # wpack2: [:, 0:1024] wkv as [8 kb, 128], [:, 1024:3072] wo as [2 c, 1024]
    wpackD = nc.declare_dram_parameter("wpack", [128, 2048], BF16,
                                       isOutput=False)
    wpack2D = nc.declare_dram_parameter("wpack2", [128, 3072], BF16,
                                        isOutput=False)
    # rope tables + causal mask packed: [:, 0:2048] cos, [:, 2048:4096] sin,
    # [:, 4096:4224] additive triangle mask (-240 where k > c)
    rpackD = nc.declare_dram_parameter("rpack", [128, 4224], BF16,
                                       isOutput=False)
    out = nc.declare_dram_parameter("out", [DM, S], BF16, isOutput=True)

    xT_v = xT.rearrange("(kb p) n -> p kb n", p=128)        # [128, 8, S]
    out_v = out.rearrange("(mb p) n -> p mb n", p=128)      # [128, 8, S]

    EXP = mybir.ActivationFunctionType.Exp
    MUL = bass.mybir.AluOpType.mult
    ADD = bass.mybir.AluOpType.add

    with tile.TileContext(nc) as tc:
        with (
            tc.tile_pool(name="consts", bufs=1) as consts,
            tc.tile_pool(name="kch", bufs=NCH) as kch_pool,
            tc.tile_pool(name="qch", bufs=NCH) as qch_pool,
            tc.tile_pool(name="ach", bufs=2 * NCH) as ach_pool,
            tc.tile_pool(name="v1p", bufs=NKB) as v1_pool,
            tc.tile_pool(name="xch", bufs=NCH) as xch_pool,
            tc.tile_pool(name="tmp", bufs=6) as tmp_pool,
            tc.tile_pool(name="pp", bufs=8) as p_pool,
            tc.tile_pool(name="rp", bufs=8) as r_pool,
            tc.tile_pool(name="ocp", bufs=4) as oc_pool,
            tc.tile_pool(name="outb", bufs=2) as outb_pool,
            tc.tile_pool(name="acc", bufs=2, space="PSUM") as acc_pool,
            tc.tile_pool(name="oac", bufs=1, space="PSUM") as oacc_pool,
            tc.tile_pool(name="sme", bufs=2, space="PSUM") as s_pool,
        ):
            # ---- constants ----
            wpack = consts.tile([128, 2048], BF16, tag="wpack")
            wpack2 = consts.tile([128, 3072], BF16, tag="wpack2")
            rpack = consts.tile([128, 4224], BF16, tag="rpack")
            ident = consts.tile([128, 128], BF16, tag="ident")

            def wq_ap(kb, lo, hi):
                return wpack[:, kb * 256 + lo: kb * 256 + hi]

            def wkv_ap(kb):
                return wpack2[:, kb * 128: (kb + 1) * 128]

            def wo_ap(c, lo, hi):
                base = 1024 + c * 1024
                return wpack2[:, base + lo: base + hi]

            def cos_ap(npart, cols):
                return rpack[0:npart, cols]

            def sin_ap(npart, cols):
                return rpack[0:npart, 2048 + cols.start: 2048 + cols.stop]

            def mask_ap():
                return rpack[:, 4096:4224]

            # spread the prologue loads over three DMA queues (sync, scalar
            # HWDGE, gpsimd SWDGE) so they don't serialize on one engine,
            # and keep the DMA count below the 8 semaphore lanes to avoid
            # false lane-reuse dependencies
            nc.scalar.dma_start(wpack[:], wpackD[:])
            nc.scalar.dma_start(wpack2[:], wpack2D[:])
            nc.gpsimd.dma_start(rpack[:], rpackD[:])
            make_identity(nc, ident[:])

            # ~3.5us of dummy back-to-back matmuls while the first DMAs are
            # in flight: flips the PE HAM clock-gate to 8/8 (2.4 GHz) before
            # the first projection, instead of running the first ~20us of
            # real matmuls at the cold 1.2 GHz default.
            if WARMUP:
                warm_ps = acc_pool.tile([128, 128], F32, tag="acc")
                for _ in range(32):
                    nc.tensor.matmul(warm_ps[:], ident[:], ident[:],
                                     start=True, stop=True)

            K_ch = []       # per-chunk K, feature-major, duplicated rows
            Q_ch = {}       # per-chunk Q, feature-major, [128, 2, SC]
            A_ch = {}       # per-chunk attn output, feature-major
            V1_kb = []      # per key-block token-major [V | 1]

            def proj_load(c0):
                cols = slice(c0 * SC, (c0 + 1) * SC)
                xc = xch_pool.tile([128, 8, SC], BF16, tag="x",
                                   name=f"x_c{c0}")
                nc.sync.dma_start(xc[:], xT_v[:, :, cols])
                return xc

            def rope(dst, src_ps, cols, npart):
                """dst[bf16] = RoPE(src_ps[f32 PSUM]) on npart partitions."""
                qsw = tmp_pool.tile([npart, SC], F32, tag="qsw")
                t1 = tmp_pool.tile([npart, SC], BF16, tag="t1")
                t2 = tmp_pool.tile([npart, SC], BF16, tag="t2")
                nc.vector.stream_shuffle(qsw[:], src_ps, XOR1)
                # fused PSUM evacuation + cos multiply
                nc.vector.scalar_tensor_tensor(
                    t1[:], src_ps, 0.0, cos_ap(npart, cols), ADD, MUL)
                nc.vector.tensor_tensor(t2[:], qsw[:], sin_ap(npart, cols), MUL)
                nc.vector.tensor_tensor(dst, t1[:], t2[:], ADD)

            def proj_q(c0, xc, ms=(0, 1), q_sb=None):
                cols = slice(c0 * SC, (c0 + 1) * SC)
                if q_sb is None:
                    q_sb = qch_pool.tile([128, 2, SC], BF16, tag="q")
                    Q_ch[c0] = q_sb
                # Q projection + RoPE, two 128-feature tiles (2 heads each)
                for m in ms:
                    q_ps = acc_pool.tile([128, SC], F32, tag="acc")
                    for kb in range(8):
                        nc.tensor.matmul(
                            q_ps[:],
                            wq_ap(kb, m * 128, (m + 1) * 128),
                            xc[:, kb, :],
                            start=(kb == 0), stop=(kb == 7),
                        )
                    rope(q_sb[:, m, :], q_ps[:], cols, 128)
                return q_sb

            def proj_kv(c0, xc):
                cols = slice(c0 * SC, (c0 + 1) * SC)
                k_sb = kch_pool.tile([128, SC], BF16, tag="k")
                K_ch.append(k_sb)
                # K (rows 0:64) and V (rows 64:128) projection
                kv_ps = acc_pool.tile([128, SC], F32, tag="acc")
                for kb in range(8):
                    nc.tensor.matmul(
                        kv_ps[:],
                        wkv_ap(kb),
                        xc[:, kb, :],
                        start=(kb == 0), stop=(kb == 7),
                    )
                # V cast FIRST on the DVE so the PE transposes (queued ahead
                # of the next projections in the FIFO) unblock before the
                # K rope finishes
                vtmp = tmp_pool.tile([128, SC], BF16, tag="vtmp")
                nc.vector.tensor_copy(vtmp[64:128, :], kv_ps[64:128, :])
                rope(k_sb[0:64, :], kv_ps[0:64, :], cols, 64)
                # duplicate K into partitions 64:128 (second PE row group)
                nc.sync.dma_start(k_sb[64:128, :], k_sb[0:64, :])
                for tb in range(4):
                    v1 = v1_pool.tile([128, 66], BF16, tag="v1")
                    V1_kb.append(v1)
                    vt_ps = acc_pool.tile([128, 64], BF16, tag="acc")
                    nc.tensor.transpose(
                        vt_ps[:], vtmp[64:128, tb * 128:(tb + 1) * 128],
                        ident[64:128, 64:128],
                    )
                    nc.vector.tensor_copy(v1[:, 0:64], vt_ps[:])
                    nc.vector.memset(v1[:, 64:65], 1.0)

            def attention_pair(c0, p, exp_dve=False):
                nkb = 4 * (c0 + 1)
                op2 = oacc_pool.tile([65, 2, SC], F32, tag="oacc")
                o0 = op2[:, 0, :]
                o1 = op2[:, 1, :]

                def av(kb, p2v, q0):
                    nc.tensor.matmul(
                        o0[:, q0:], V1_kb[kb][:, 0:65], p2v(0, q0),
                        start=(kb == 0), stop=(kb == nkb - 1),
                    )
                    nc.tensor.matmul(
                        o1[:, q0:], V1_kb[kb][:, 0:65], p2v(1, q0),
                        start=(kb == 0), stop=(kb == nkb - 1),
                    )

                for kb in range(nkb):
                    kc = K_ch[kb // 4]
                    kcols = slice((kb % 4) * 128, (kb % 4 + 1) * 128)
                    s2 = s_pool.tile([128, 2, SC], F32, tag="s2")
                    nc.tensor.matmul(
                        s2[:, 0, :],
                        kc[0:64, kcols],
                        Q_ch[c0][0:64, p, :],
                        start=True, stop=True,
                    )
                    nc.tensor.matmul(
                        s2[:, 1, :],
                        kc[64:128, kcols],
                        Q_ch[c0][64:128, p, :],
                        start=True, stop=True,
                        tile_position=(64, 0),
                    )
                    # on diagonal blocks, only q-cols >= q0 are live
                    j = kb - (nkb - 4)
                    q0 = 128 * j if j >= 0 else 0
                    if j >= 0 and MASK_PE:
                        # additive causal mask on the triangular 128-col
                        # sub-block via PE accumulate (cols >= q0+128 of this
                        # key block are fully causal-valid)
                        for h in range(2):
                            nc.tensor.matmul(
                                s2[:, h, q0:q0 + 128],
                                ident[:], mask_ap(),
                                start=False, stop=True, skip_group_check=True,
                            )
                    if exp_dve and kb % 2 == 1 and j < 0:
                        # Schraudolph exp on DVE: bf16 bits of exp(SCALE*s)
                        # via int16 linear map (splits the exp stream between
                        # the saturated ACT engine and the DVE)
                        p2i = p_pool.tile([128, 2, SC], mybir.dt.int16,
                                          tag="p2")
                        nc.vector.tensor_scalar(
                            p2i[:, :, q0:], s2[:, :, q0:],
                            SCH_A, SCH_B, MUL, ADD)

                        def p2v(sl_h, lo, hi=SC):
                            return p2i[:, sl_h, lo:hi].bitcast(BF16)
                    else:
                        p2 = p_pool.tile([128, 2, SC], BF16, tag="p2")
                        nc.scalar.activation(p2[:, :, q0:], s2[:, :, q0:],
                                             EXP, scale=SCALE)

                        def p2v(sl_h, lo, hi=SC):
                            return p2[:, sl_h, lo:hi]
                    if j >= 0 and not MASK_PE:
                        nc.vector.tensor_tensor(
                            p2v(slice(None), q0, q0 + 128),
                            p2v(slice(None), q0, q0 + 128),
                            mask_ap().unsqueeze(1).to_broadcast([128, 2, 128]),
                            MUL)
                    av(kb, p2v, q0)
                # evacuate PSUM accumulator to SBUF bf16 (frees the banks).
                # ACT does this: it reads PSUM faster than the DVE and the
                # vector engine is the more loaded of the two.
                oc2 = oc_pool.tile([65, 2, SC], BF16, tag="oc")
                nc.scalar.copy(oc2[:], op2[:])
                return oc2

            def divide_pair(a_tile, oc2):
                # reshape the [1, 2*512] sums row to [64, 16] so the
                # reciprocal runs on many DVE lanes instead of one
                rsum = r_pool.tile([64, 16], BF16, tag="rsum")
                nc.sync.dma_start(
                    rsum[:],
                    oc2[64:65, :, :].rearrange("o hh (a n) -> o (hh a) n", a=32))
                # bf16 reciprocal directly: the value feeds a bf16 multiply
                # anyway, so the f32 intermediate (and its extra DVE op +
                # semaphore hop on this latency chain) adds nothing
                rrecs = r_pool.tile([64, 16], BF16, tag="rrecs")
                with nc.allow_low_precision("bf16 softmax denominators"):
                    nc.vector.reciprocal(rrecs[:], rsum[:])
                rrec = r_pool.tile([1, 2, SC], BF16, tag="rrec")
                nc.sync.dma_start(
                    rrec[0:1, :, :].rearrange("o hh (a n) -> o (hh a) n", a=32),
                    rrecs[:])
                # one broadcast covers both heads' reciprocal rows: the
                # second head's multiply no longer waits on a second
                # gpsimd launch
                bc = r_pool.tile([64, 2, SC], BF16, tag="bc")
                nc.gpsimd.partition_broadcast(bc[:], rrec[0:1, :, :])
                nc.vector.tensor_tensor(
                    a_tile[0:64, :], oc2[0:64, 0, :], bc[:, 0, :], MUL)
                tb = r_pool.tile([64, SC], BF16, tag="tb")
                nc.vector.tensor_tensor(
                    tb[:], oc2[0:64, 1, :], bc[:, 1, :], MUL)
                # move to partitions 64:128 (DMA crosses partitions)
                nc.sync.dma_start(a_tile[64:128, :], tb[:])

            def out_proj(c0):
                ncols = slice(c0 * SC, (c0 + 1) * SC)
                ob = outb_pool.tile([128, 8, SC], BF16, tag="ob")
                for mb in range(8):
                    o_ps = acc_pool.tile([128, SC], F32, tag="acc")
                    for c in range(2):
                        nc.tensor.matmul(
                            o_ps[:],
                            wo_ap(c, mb * 128, (mb + 1) * 128),
                            A_ch[c0][c][:, :],
                            start=(c == 0), stop=(c == 1),
                        )
                    nc.vector.tensor_copy(ob[:, mb, :], o_ps[:])
                    if mb == 3:
                        nc.sync.dma_start(out_v[:, 0:4, ncols], ob[:, 0:4, :])
                nc.sync.dma_start(out_v[:, 4:8, ncols], ob[:, 4:8, :])

            X_ch = [proj_load(c) for c in range(NCH)]
            # chunk 0 prologue: emit the m0 Q-tile first, then the KV
            # projection, so the PE works on KV while the DVE ropes m0 and
            # attention(0, 0) (which only needs m0 + K + V) starts early
            q0_sb = proj_q(0, X_ch[0], ms=(0,))
            proj_kv(0, X_ch[0])
            proj_q(0, X_ch[0], ms=(1,), q_sb=q0_sb)
            pending_outproj = None
            for c0 in range(NCH):
                a_pair = [ach_pool.tile([128, SC], BF16, tag="a",
                                        name=f"a_c{c0}p{ii}")
                          for ii in range(2)]
                A_ch[c0] = a_pair
                oc0 = attention_pair(c0, 0)
                divide_pair(a_pair[0], oc0)
                if c0 + 1 < NCH:
                    proj_q(c0 + 1, X_ch[c0 + 1])  # fills PE bubbles
                elif pending_outproj is not None:
                    out_proj(pending_outproj)     # last chunk: drain backlog
                    pending_outproj = None
                oc1 = attention_pair(c0, 1, exp_dve=(c0 == NCH - 1))
                if c0 + 1 < NCH:
                    proj_kv(c0 + 1, X_ch[c0 + 1])
                else:
                    # tail keep-warm: hold the PE HAM at 8/8 through the
                    # divide/out-proj drain so the final matmuls run at
                    # 2.4 GHz. Reading oc1 makes these matmuls depend on the
                    # final evacuation, pinning them into the drain window
                    # (ident-only dummies would be scheduled much earlier).
                    warm2 = acc_pool.tile([128, SC], F32, tag="acc")
                    for _ in range(20):
                        nc.tensor.matmul(warm2[:], ident[0:65, :],
                                         oc1[0:65, 0, :],
                                         start=True, stop=True)
                divide_pair(a_pair[1], oc1)
                if pending_outproj is not None:
                    out_proj(pending_outproj)
                pending_outproj = c0
            out_proj(NCH - 1)

    nc.compile()
    return nc


def shard_inputs(x, wq, wk, wv, wo, freqs_cos, freqs_sin):
    """Build the 8 per-core input maps (host-side layout prep)."""
    bf = ml_dtypes.bfloat16
    x = np.ascontiguousarray(np.asarray(x, dtype=np.float32))
    wq = np.asarray(wq, dtype=np.float32)
    wk = np.asarray(wk, dtype=np.float32)
    wv = np.asarray(wv, dtype=np.float32)
    wo = np.asarray(wo, dtype=np.float32)
    cos = np.asarray(freqs_cos, dtype=np.float32)   # [S, 32]
    sin = np.asarray(freqs_sin, dtype=np.float32)

    rope_cos = np.repeat(cos.T, 2, axis=0)          # [64, S]
    rope_sin = np.repeat(sin.T, 2, axis=0)
    rope_sin[0::2, :] *= -1.0                       # row 2i: -sin_i, 2i+1: +sin_i

    # causal mask for the triangular diagonal sub-block: additive (-240 where
    # k > c; exp underflows to 0) for the PE path, multiplicative otherwise
    kk = np.arange(128)[:, None]
    cc = np.arange(128)[None, :]
    if MASK_PE:
        mask01 = np.where(kk > cc, -240.0, 0.0)
    else:
        mask01 = (kk <= cc).astype(np.float32)

    # rpack: [cos | sin | mask], each table duplicated to 128 partitions
    rpack = np.concatenate([
        np.tile(rope_cos, (2, 1)),                  # [128, S]
        np.tile(rope_sin, (2, 1)),                  # [128, S]
        mask01,                                     # [128, 128]
    ], axis=1).astype(bf)

    in_maps = []
    for core in range(N_CORES):
        b, r = divmod(core, TPG)
        xT = np.ascontiguousarray(x[b].T)                         # [DM, S]
        wq_s = wq[r * KFEAT:(r + 1) * KFEAT]                      # [256, DM]
        wk_s = wk[r * HD:(r + 1) * HD]                            # [64, DM]
        wv_s = wv[r * HD:(r + 1) * HD]
        wkvT = np.concatenate([wk_s, wv_s], axis=0).T             # [DM, 128]
        wqT = wq_s.T                                              # [DM, 256]
        woT = wo[:, r * KFEAT:(r + 1) * KFEAT].T                  # [256, DM]
        # wq [128, 8kb, 256]; wkv [128, 8kb, 128]; wo [128, 2c, 1024]
        wq_l = wqT.reshape(8, 128, KFEAT).transpose(1, 0, 2).reshape(128, 2048)
        wkv_l = wkvT.reshape(8, 128, 128).transpose(1, 0, 2).reshape(128, 1024)
        wo_l = woT.reshape(2, 128, DM).transpose(1, 0, 2).reshape(128, 2048)
        wpack2 = np.concatenate([wkv_l, wo_l], axis=1)            # [128, 3072]
        in_maps.append({
            "xT": xT.astype(bf),
            "wpack": np.ascontiguousarray(wq_l).astype(bf),
            "wpack2": np.ascontiguousarray(wpack2).astype(bf),
            "rpack": rpack,
        })
    return in_maps


def unshard(results):
    """Sum TP partials per batch and transpose back to [B, S, DM]."""
    out = np.empty((B, S, DM), dtype=np.float32)
    for b in range(B):
        acc = results[b * TPG]["out"].astype(np.float32)
        for r in range(1, TPG):
            acc = acc + results[b * TPG + r]["out"].astype(np.float32)
        out[b] = acc.T
    return out


def kernel(**inputs):
    global LAST_RESULTS, _NC_CACHE
    if _NC_CACHE is None:
        _NC_CACHE = build_nc()
    in_maps = shard_inputs(**inputs)
    LAST_RESULTS = run_bass_kernel_spmd(_NC_CACHE, in_maps, list(range(N_CORES)))
    return unshard(LAST_RESULTS.results)
